# revision 10
# baseline (speedup 1.0000x reference)
"""nn_GatedRecurrentBlock — hand-written Bass/Tile kernel for 8x Trainium2 cores.

Strategy (data-parallel over batch, 1024 rows/core, 2 row-blocks of 512):
  - Host folds the attention v-proj + out-proj into one matrix (softmax over a
    single key == 1, so attn == v), folds g1/g2 into the weight matrices, pads
    HIDDEN 5324 -> 5376, and pre-packs every weight into consumption-ordered
    [Mtiles, 128, Ktiles, 128] bf16 lhsT tiles (contiguous DMA).
  - Activations live feature-major ([feat, row]) on chip so no on-device
    transposes are needed; host ships x/state pre-transposed bf16 and
    re-transposes the fp32 output.
  - RMSNorm: squares on ScalarE, cross-partition reduce via ones-matmul on
    TensorE, per-row 1/norm broadcast via gpsimd partition_broadcast, scale
    folded into PSUM eviction (attn) or a single tensor_tensor (ffn).
"""

import numpy as np
import ml_dtypes
import bass_rust

import concourse.bass as bass
import concourse.mybir as mybir
import concourse.tile as tile
from concourse.bass_utils import run_bass_kernel_spmd

P = 128
DIM = 2048
KC = DIM // P              # 16 feature chunks
HID = 5324
HIDP = 5376                # padded hidden
MH = HIDP // P             # 42 hidden chunks
BATCH = 8192
NCORES = 8
RPC = BATCH // NCORES      # 1024 rows per core
R = 512                    # rows per block (one PSUM bank of fp32)
NB = RPC // R              # 2 blocks
EPS = 1e-6
ISQ = DIM ** -0.5
SQH = 0.7071

BF16 = mybir.dt.bfloat16
F32 = mybir.dt.float32
nbf = ml_dtypes.bfloat16

AF = mybir.ActivationFunctionType
ALU = mybir.AluOpType


def _pack_lhsT(w):
    """[K, M] fp32 -> [M//P, P, K//P, P] bf16, tile-contiguous in consumption order."""
    K, M = w.shape
    return np.ascontiguousarray(
        w.astype(nbf).reshape(K // P, P, M // P, P).transpose(2, 1, 0, 3))


def _opname(inst):
    try:
        return inst.opcode.name
    except AttributeError:
        return str(inst.opcode)


def _split_waits(nc, caps={}, cap_default=1):
    """This neuronxcc build caps sync-wait commands at 1 per instruction.
    Tile's wait assignment can exceed that; spill excess waits onto preceding
    same-engine NOPs (sequencers execute waits in program order, so this is
    semantics-preserving)."""
    for f in nc.m.functions:
        for bb in f.blocks:
            insts = list(bb.instructions)
            out = []
            changed = False
            for inst in insts:
                si = inst.sync_info
                waits = list(si.on_wait) if si is not None and si.on_wait else []
                cap = caps.get(_opname(inst), cap_default)
                if len(waits) > cap:
                    changed = True
                    keep = waits[len(waits) - cap:] if cap > 0 else []
                    spill = waits[: len(waits) - cap] if cap > 0 else waits
                    for w in spill:
                        nop = mybir.InstNoOp(name=f"I-{nc.next_id()}", ins=[], outs=[])
                        nop.engine = inst.engine
                        nop.sync_info = bass_rust.SyncInfo(on_wait=[w], on_update=[])
                        nc.register_instruction(nop, overwrite=True)
                        out.append(nop)
                    si.on_wait = keep
                    inst.sync_info = si
                out.append(inst)
            if changed:
                bb.instructions = out
    return nc


def build_program(use_bvo: bool, use_gb: bool) -> bass.Bass:
    nc = bass.Bass("TRN2", debug=False)

    xT = nc.dram_tensor("xT", [KC, P, RPC], BF16, kind="ExternalInput")
    sT = nc.dram_tensor("sT", [KC, P, RPC], BF16, kind="ExternalInput")
    wvo = nc.dram_tensor("wvo", [KC, P, KC, P], BF16, kind="ExternalInput")
    w1 = nc.dram_tensor("w1", [MH, P, KC, P], BF16, kind="ExternalInput")
    w2 = nc.dram_tensor("w2", [MH, P, KC, P], BF16, kind="ExternalInput")
    w3 = nc.dram_tensor("w3", [KC, P, MH, P], BF16, kind="ExternalInput")
    wg = nc.dram_tensor("wg", [KC, P, 2 * KC, P], BF16, kind="ExternalInput")
    if use_bvo:
        bvo = nc.dram_tensor("bvo", [P, KC], F32, kind="ExternalInput")
    if use_gb:
        gb = nc.dram_tensor("gb", [P, KC], F32, kind="ExternalInput")
    yT = nc.dram_tensor("yT", [KC, P, RPC], F32, kind="ExternalOutput")

    with tile.TileContext(nc) as tc:
        with (
            tc.tile_pool(name="const", bufs=1) as constp,
            tc.tile_pool(name="pA", bufs=1) as pA,
            tc.tile_pool(name="pB", bufs=1) as pB,
            tc.tile_pool(name="pC", bufs=1) as pC,
            tc.tile_pool(name="pS", bufs=1) as pS,
            tc.tile_pool(name="pG", bufs=1) as pG,
            tc.tile_pool(name="pW", bufs=2) as pW,
            tc.tile_pool(name="small", bufs=3) as psm,
            tc.tile_pool(name="mm", bufs=6, space="PSUM") as pmm,
            tc.tile_pool(name="ssp", bufs=2, space="PSUM") as pss,
        ):
            ones = constp.tile([P, 1], BF16)
            nc.vector.memset(ones, 1.0)
            ones_row = constp.tile([1, P], F32)
            nc.vector.memset(ones_row, 1.0)
            if use_bvo:
                bvo_sb = constp.tile([P, KC], F32)
                nc.sync.dma_start(out=bvo_sb, in_=bvo.ap())
            if use_gb:
                gb_sb = constp.tile([P, KC], F32)
                nc.sync.dma_start(out=gb_sb, in_=gb.ap())

            for b in range(NB):
                rows = slice(b * R, (b + 1) * R)

                # ---- load inputs (feature-major bf16) ----
                xt = pA.tile([P, KC, R], BF16, tag="A")
                st = pS.tile([P, KC, R], BF16, tag="S")
                nc.sync.dma_start(out=xt, in_=xT.ap()[:, :, rows].rearrange("c p r -> p c r"))
                nc.sync.dma_start(out=st, in_=sT.ap()[:, :, rows].rearrange("c p r -> p c r"))

                # ---- h = (x + state) * sqrt(1/2); res := h ----
                res = pB.tile([P, KC, R], BF16, tag="B")
                nc.vector.tensor_add(out=res, in0=xt, in1=st)
                nc.vector.tensor_scalar_mul(res, res, SQH)

                # ---- rmsnorm 1: ssq over features via ones-matmul ----
                ss1 = pss.tile([1, R], F32, tag="ss")
                for c in range(KC):
                    hsq = psm.tile([P, R], BF16, tag="hsq")
                    nc.scalar.activation(out=hsq, in_=res[:, c], func=AF.Square)
                    nc.tensor.matmul(ss1, ones, hsq, start=(c == 0), stop=(c == KC - 1))
                rn1 = psm.tile([1, R], F32, tag="rn")
                nc.scalar.activation(out=rn1, in_=ss1, func=AF.Sqrt)
                nc.vector.tensor_scalar_max(rn1, rn1, EPS)
                nc.vector.reciprocal(rn1, rn1)
                nc.vector.tensor_scalar_mul(rn1, rn1, ISQ)
                psb1 = pmm.tile([P, R], F32, tag="ps")
                nc.tensor.matmul(psb1, ones_row, rn1, start=True, stop=True)
                bc1 = psm.tile([P, R], F32, tag="bc1", bufs=2)
                nc.scalar.copy(bc1, psb1)

                # ---- attention (fused v+out proj); h2 = res + rn1*psum + bvo ----
                # norm2 squares interleave with the evictions.
                h2 = pC.tile([P, KC, R], BF16, tag="C")
                ss2 = pss.tile([1, R], F32, tag="ss")
                for m in range(KC):
                    wt = pW.tile([P, KC, P], BF16, tag="wvo")
                    nc.sync.dma_start(out=wt, in_=wvo.ap()[m])
                    ps = pmm.tile([P, R], F32, tag="ps")
                    for k in range(KC):
                        nc.tensor.matmul(ps, wt[:, k], res[:, k],
                                         start=(k == 0), stop=(k == KC - 1))
                    tb = psm.tile([P, R], BF16, tag="tb")
                    nc.vector.tensor_tensor(tb, ps, bc1, ALU.mult)
                    if use_bvo:
                        nc.vector.tensor_scalar_add(tb, tb, bvo_sb[:, m:m + 1])
                    nc.vector.tensor_add(out=h2[:, m], in0=tb, in1=res[:, m])
                    hsq = psm.tile([P, R], BF16, tag="hsq")
                    nc.scalar.activation(out=hsq, in_=h2[:, m], func=AF.Square)
                    nc.tensor.matmul(ss2, ones, hsq, start=(m == 0), stop=(m == KC - 1))

                # ---- rmsnorm 2 -> hn2 (bf16) ----
                rn2 = psm.tile([1, R], F32, tag="rn")
                nc.scalar.activation(out=rn2, in_=ss2, func=AF.Sqrt)
                nc.vector.tensor_scalar_max(rn2, rn2, EPS)
                nc.vector.reciprocal(rn2, rn2)
                nc.vector.tensor_scalar_mul(rn2, rn2, ISQ)
                psb2 = pmm.tile([P, R], F32, tag="ps")
                nc.tensor.matmul(psb2, ones_row, rn2, start=True, stop=True)
                bc2 = psm.tile([P, R], BF16, tag="bc2", bufs=2)
                nc.scalar.copy(bc2, psb2)
                hn2 = pA.tile([P, KC, R], BF16, tag="A")
                nc.vector.tensor_tensor(
                    hn2, h2, bc2[:, None, :].to_broadcast((P, KC, R)), ALU.mult)

                # ---- FFN up: g = silu(hn2 @ w1T) * (hn2 @ w2T) ----
                g = pG.tile([P, MH, R], BF16, tag="G")
                for m in range(MH):
                    wt1 = pW.tile([P, KC, P], BF16, tag="w1")
                    nc.sync.dma_start(out=wt1, in_=w1.ap()[m])
                    wt2 = pW.tile([P, KC, P], BF16, tag="w2")
                    nc.sync.dma_start(out=wt2, in_=w2.ap()[m])
                    psa = pmm.tile([P, R], F32, tag="ps")
                    for k in range(KC):
                        nc.tensor.matmul(psa, wt1[:, k], hn2[:, k],
                                         start=(k == 0), stop=(k == KC - 1))
                    psb = pmm.tile([P, R], F32, tag="ps")
                    for k in range(KC):
                        nc.tensor.matmul(psb, wt2[:, k], hn2[:, k],
                                         start=(k == 0), stop=(k == KC - 1))
                    sg = psm.tile([P, R], BF16, tag="sg")
                    nc.scalar.activation(out=sg, in_=psa, func=AF.Sigmoid)
                    sa = psm.tile([P, R], BF16, tag="sa")
                    nc.vector.tensor_tensor(sa, sg, psa, ALU.mult)
                    nc.vector.tensor_tensor(g[:, m], sa, psb, ALU.mult)

                # ---- FFN down + residual: cand = h2 + g @ w3T ----
                cand = pB.tile([P, KC, R], BF16, tag="B")
                for m in range(KC):
                    wt3 = pW.tile([P, MH, P], BF16, tag="w3")
                    nc.sync.dma_start(out=wt3, in_=w3.ap()[m])
                    ps = pmm.tile([P, R], F32, tag="ps")
                    for k in range(MH):
                        nc.tensor.matmul(ps, wt3[:, k], g[:, k],
                                         start=(k == 0), stop=(k == MH - 1))
                    nc.vector.tensor_add(out=cand[:, m], in0=ps, in1=h2[:, m])

                # ---- gate + blend: y = state + sigmoid(gate)*(cand - state) ----
                for m in range(KC):
                    wtg = pW.tile([P, 2 * KC, P], BF16, tag="wg")
                    nc.sync.dma_start(out=wtg, in_=wg.ap()[m])
                    ps = pmm.tile([P, R], F32, tag="ps")
                    for k in range(KC):
                        nc.tensor.matmul(ps, wtg[:, k], cand[:, k],
                                         start=(k == 0), stop=False)
                    for k in range(KC):
                        nc.tensor.matmul(ps, wtg[:, KC + k], st[:, k],
                                         start=False, stop=(k == KC - 1))
                    z = psm.tile([P, R], BF16, tag="z")
                    nc.scalar.activation(out=z, in_=ps, func=AF.Sigmoid,
                                         bias=(gb_sb[:, m:m + 1] if use_gb else 0.0))
                    d = psm.tile([P, R], BF16, tag="d")
                    nc.vector.tensor_tensor(d, cand[:, m], st[:, m], ALU.subtract)
                    u = psm.tile([P, R], BF16, tag="u")
                    nc.vector.tensor_tensor(u, z, d, ALU.mult)
                    y = psm.tile([P, R], F32, tag="y")
                    nc.vector.tensor_add(out=y, in0=u, in1=st[:, m])
                    nc.sync.dma_start(out=yT.ap()[m][:, rows], in_=y)

    return _split_waits(nc)


_prog_cache: dict = {}


def _get_prog(use_bvo: bool, use_gb: bool) -> bass.Bass:
    key = (use_bvo, use_gb)
    if key not in _prog_cache:
        _prog_cache[key] = build_program(use_bvo, use_gb)
    return _prog_cache[key]


def prepare_inputs(x, state, g1, g2, in_proj_w, in_proj_b, out_proj_w, out_proj_b,
                   w1, w2, w3, gate_w, gate_b):
    """Host-side folding/packing. Returns (in_maps, use_bvo, use_gb)."""
    f32 = np.float32
    x = np.asarray(x, f32); state = np.asarray(state, f32)
    g1 = np.asarray(g1, f32); g2 = np.asarray(g2, f32)
    in_proj_w = np.asarray(in_proj_w, f32); in_proj_b = np.asarray(in_proj_b, f32)
    out_proj_w = np.asarray(out_proj_w, f32); out_proj_b = np.asarray(out_proj_b, f32)
    w1 = np.asarray(w1, f32); w2 = np.asarray(w2, f32); w3 = np.asarray(w3, f32)
    gate_w = np.asarray(gate_w, f32); gate_b = np.asarray(gate_b, f32)

    wv = in_proj_w[2 * DIM:]
    bv = in_proj_b[2 * DIM:]
    # attn == v (softmax over one key); fold v-proj + out-proj (and g1) together
    W_vo = (wv * g1[None, :]).T @ out_proj_w.T          # [K=2048, M=2048]
    b_vo = bv @ out_proj_w.T + out_proj_b               # [2048]
    W1 = np.zeros((DIM, HIDP), f32); W1[:, :HID] = (w1 * g2[None, :]).T
    W2 = np.zeros((DIM, HIDP), f32); W2[:, :HID] = (w2 * g2[None, :]).T
    W3 = np.zeros((HIDP, DIM), f32); W3[:HID] = w3.T
    WG = np.concatenate([gate_w[:, :DIM].T, gate_w[:, DIM:].T], axis=0)  # [4096, 2048]

    weights = {
        "wvo": _pack_lhsT(W_vo),
        "w1": _pack_lhsT(W1),
        "w2": _pack_lhsT(W2),
        "w3": _pack_lhsT(W3),
        "wg": _pack_lhsT(WG),
    }
    use_bvo = bool(np.any(b_vo))
    use_gb = bool(np.any(gate_b))
    if use_bvo:
        weights["bvo"] = np.ascontiguousarray(b_vo.reshape(KC, P).T)
    if use_gb:
        weights["gb"] = np.ascontiguousarray(gate_b.reshape(KC, P).T)

    in_maps = []
    for c in range(NCORES):
        rs = slice(c * RPC, (c + 1) * RPC)
        m = dict(weights)
        m["xT"] = np.ascontiguousarray(x[rs].astype(nbf).T).reshape(KC, P, RPC)
        m["sT"] = np.ascontiguousarray(state[rs].astype(nbf).T).reshape(KC, P, RPC)
        in_maps.append(m)
    return in_maps, use_bvo, use_gb


def run(inputs: dict, trace: bool = False, trace_cores=None):
    in_maps, use_bvo, use_gb = prepare_inputs(**inputs)
    nc = _get_prog(use_bvo, use_gb)
    res = run_bass_kernel_spmd(
        nc, in_maps, core_ids=list(range(NCORES)),
        trace=trace, trace_cores=trace_cores)
    out = np.empty((BATCH, DIM), np.float32)
    for c in range(NCORES):
        yt = res.results[c]["yT"].reshape(DIM, RPC)
        out[c * RPC:(c + 1) * RPC] = yt.T
    return out, res


def kernel(**inputs) -> np.ndarray:
    out, _ = run(inputs, trace=False)
    return out
